# revision 16
# baseline (speedup 1.0000x reference)
"""Trainium2 Bass kernel for nn_ActorCritic (LSTM with done-resets + heads).

Sharding: TIME-sharded. The done-resets (p=0.5/step) make state older than
~30 steps irrelevant, so each core processes a K-step span: a warmup from
zero state re-synchronizes (h,c) exactly (every env is guaranteed a reset
inside the warmup window by construction), then the owned steps. K and the
7 segment boundaries are computed AT RUNTIME from the actual done data
(binary search for the smallest uniform span such that greedy boundary
placement covers T=512), so the result is exact for any input. Each core
sees the FULL batch B=256; no collectives.

Host-side marshalling (not compute): x is cast to bf16 and pre-transposed to
x^T_aug [294, K*256] with row 292 = 1.0 (folds gate bias into the xW GEMM)
and row 293 = done_t scaled by -30 into the f-gate column block (sigmoid(f)
-> 0 on reset steps, which zeroes the c-history exactly like the reference's
c*(1-d) mask). Gate blocks are reordered [o,i,f,g] and the g block (weights
+ bias) is pre-doubled so one sigmoid yields sigma(2g), with
tanh(g) = 2*sigma(2g)-1 recovered inside the fused DVE tail.

Device per core, per step (B=256 as two interleaved 128-wide half-batches so
the two serial recurrence chains hide each other's latency):
  - xW GEMM (3 K-tiles x 4 gates, 256-col pieces covering both halves)
    streams ~2 steps ahead directly into the step's [128,1024] PSUM tile
    (2 banks; one start=True per 2KB zero region); W_hh matmuls accumulate
    on top. No SBUF xw staging, no fold matmul, no PSUM->SBUF copies.
  - ACT per half: sigmoid over [i,f,g] (critical path), sigmoid over [o]
    (off-path), tanh(c_new).
  - DVE per half (bf16, 2x mode): t2=sig_f*c; u=(sig_g'-0.5)*sig_i;
    c_new=2u+t2 (== sig_f*c + sig_i*tanh(g)); hm=som*tanh(c).
  - Pool per half (off the critical chain): som=sig_o*m,
    h=sig_o*tanh(c) into the bf16 history consumed by the heads.
  - Heads ride at the top of each step: 2 matmuls (16 cols) + fused
    bias-add/copy on DVE + one DMA per step into a padded [K*256,16]
    output (host strips the pad).
"""

import sys
from contextlib import ExitStack

import numpy as np

sys.path.insert(0, "/opt/trn_rl_repo")

# Problem constants (hardcoded per harness contract).
T = 512
B = 256
NCORES = 8
IN = 292
H = 128
A = 12
NOUT = 13
HB = 128  # half-batch width

INA = IN + 2  # +ones row (bias), +done row (f-gate kill)
KSPLITS = [(0, 128), (128, 128), (256, INA - 256)]
MAXTCH = 26  # max steps per input chunk (SBUF budget)


def _chunks(K):
    nch = -(-K // MAXTCH)
    base = K // nch
    rem = K - base * nch
    return [base + (1 if i < rem else 0) for i in range(nch)]


def build_nc(K):
    import concourse.bass as bass
    import concourse.tile as tile
    from concourse import bacc, mybir

    f32 = mybir.dt.float32
    bf16 = mybir.dt.bfloat16
    AF = mybir.ActivationFunctionType
    OP = mybir.AluOpType

    tchs = _chunks(K)
    NCH = len(tchs)
    coff = [0]
    for tc_ in tchs:
        coff.append(coff[-1] + tc_)
    step_chunk = []
    for ch, tc_ in enumerate(tchs):
        step_chunk += [ch] * tc_

    nc = bacc.Bacc("TRN2", target_bir_lowering=False, debug=False)

    # ---- I/O (all per-core slices prepared by host) ----
    xt_d = nc.dram_tensor("xt", [INA, K * B], bf16, kind="ExternalInput").ap()
    m_d = nc.dram_tensor("m", [128, K * B], bf16, kind="ExternalInput").ap()
    h0_d = nc.dram_tensor("h0", [128, B], bf16, kind="ExternalInput").ap()
    c0_d = nc.dram_tensor("c0", [128, B], bf16, kind="ExternalInput").ap()
    wih_d = nc.dram_tensor("wih", [INA, 512], bf16, kind="ExternalInput").ap()
    whh_d = nc.dram_tensor("whh", [128, 512], bf16, kind="ExternalInput").ap()
    wcat_d = nc.dram_tensor("wcat", [128, 16], bf16, kind="ExternalInput").ap()
    bhd_d = nc.dram_tensor("bhd", [128, 32], f32, kind="ExternalInput").ap()
    out_d = nc.dram_tensor("out", [K * B, 16], f32, kind="ExternalOutput").ap()

    with tile.TileContext(nc) as tc, ExitStack() as ctx:
        cst = ctx.enter_context(tc.tile_pool(name="cst", bufs=1))
        big = ctx.enter_context(tc.tile_pool(name="big", bufs=1))
        xtp = ctx.enter_context(tc.tile_pool(name="xtp", bufs=2))
        mp = ctx.enter_context(tc.tile_pool(name="mp", bufs=2))
        wk = ctx.enter_context(tc.tile_pool(name="wk", bufs=3))
        pg_pool = ctx.enter_context(tc.tile_pool(name="pg", bufs=3, space="PSUM"))
        php = ctx.enter_context(tc.tile_pool(name="ph", bufs=2, space="PSUM"))

        # ---- persistent tiles ----
        wih_sb = [cst.tile([sz, 512], bf16, tag=f"wih{k}", name=f"wih{k}")
                  for k, (_, sz) in enumerate(KSPLITS)]
        whh_sb = cst.tile([128, 512], bf16, tag="whh", name="whh")
        wcat_sb = cst.tile([128, 16], bf16, tag="wcat", name="wcat")
        bhd_sb = cst.tile([128, 32], f32, tag="bhd", name="bhd")
        h0_sb = cst.tile([128, B], bf16, tag="h0", name="h0")
        c0_sb = cst.tile([128, B], bf16, tag="c0", name="c0")
        hs_all = big.tile([128, K * B], bf16, tag="hs", name="hs")

        for k, (off, sz) in enumerate(KSPLITS):
            nc.sync.dma_start(out=wih_sb[k][:, :], in_=wih_d[off:off + sz, :])
        nc.sync.dma_start(out=whh_sb[:, :], in_=whh_d[:, :])
        nc.sync.dma_start(out=wcat_sb[:, :], in_=wcat_d[:, :])
        nc.sync.dma_start(out=bhd_sb[:, :], in_=bhd_d[:, :])
        nc.sync.dma_start(out=h0_sb[:, :], in_=h0_d[:, :])
        nc.sync.dma_start(out=c0_sb[:, :], in_=c0_d[:, :])

        # ---- input chunk DMAs ----
        xts = {}
        mts = {}

        def load_chunk(ch):
            if ch >= NCH:
                return
            cols = slice(coff[ch] * B, coff[ch + 1] * B)
            n = tchs[ch] * B
            tiles = []
            for k, (off, sz) in enumerate(KSPLITS):
                xt = xtp.tile([sz, MAXTCH * B], bf16, tag=f"xt{k}", name=f"xt{k}")
                nc.sync.dma_start(out=xt[:, 0:n], in_=xt_d[off:off + sz, cols])
                tiles.append(xt)
            xts[ch] = tiles
            mt = mp.tile([128, MAXTCH * B], bf16, tag="mt", name="mt")
            nc.sync.dma_start(out=mt[:, 0:n], in_=m_d[:, cols])
            mts[ch] = mt

        load_chunk(0)
        load_chunk(1)

        # ---- xW GEMM straight into the step's PSUM tile ----
        # One [128, 4*256] tile per step (2 banks; cols = slot*256 + b).
        # Slot order [o, i, f, g]. Each piece covers BOTH halves (256 moving
        # cols per stationary load). ONE start per 2KB zero region (slot 0 /
        # slot 2 first k-piece); every address's first write in the group
        # auto-zeroes, so later slots accumulate correctly.
        psum_tiles = {}
        PIECES = [(slot, k) for slot in range(4) for k in range(len(KSPLITS))]

        def emit_xw(t, pieces):
            if t >= K:
                return
            if t in psum_tiles:
                pg = psum_tiles[t]
            else:
                pg = pg_pool.tile([128, 1024], f32, tag="pg", name="pg")
                psum_tiles[t] = pg
            tiles = xts[step_chunk[t]]
            c0_ = (t - coff[step_chunk[t]]) * B
            for slot, k in pieces:
                off, sz = KSPLITS[k]
                nc.tensor.matmul(
                    pg[:, slot * B:slot * B + B],
                    wih_sb[k][0:sz, slot * 128:(slot + 1) * 128],
                    tiles[k][0:sz, c0_:c0_ + B],
                    start=(slot in (0, 2) and k == 0), stop=False)

        emit_xw(0, PIECES)
        emit_xw(1, PIECES)

        hm_prev = [h0_sb[:, 0:HB], h0_sb[:, HB:B]]
        c_prev = [c0_sb[:, 0:HB], c0_sb[:, HB:B]]

        def emit_heads(t):
            ph = php.tile([128, 512], f32, tag="ph", name="ph")
            for hb in range(2):
                nc.tensor.matmul(ph[:, hb * 16:hb * 16 + 16],
                                 hs_all[:, t * B + hb * HB:t * B + hb * HB + HB],
                                 wcat_sb[:, :], start=(hb == 0), stop=(hb == 1))
            ob = wk.tile([128, 32], f32, tag="ob", name="ob")
            nc.vector.scalar_tensor_tensor(
                ob[:, :], ph[:, 0:32], 1.0, bhd_sb[:, :], OP.mult, OP.add)
            nc.sync.dma_start(
                out=out_d[t * B:(t + 1) * B, :].rearrange(
                    "(a p) s -> p a s", a=2, p=128),
                in_=ob[:, :].rearrange("p (a s) -> p a s", a=2))

        # ---- the recurrence ----
        # Slot order [o, i, f, g]: sigma over slots 1:4 ([i,f,g]) is the only
        # ACT op on the critical path; sigma(o) runs off-path for som/hs.
        for t in range(K):
            if t > 0 and t - 1 in coff:
                load_chunk(coff.index(t - 1) + 2)
            mt = mts[step_chunk[t]]
            mc0 = (t - coff[step_chunk[t]]) * B
            if t > 1:
                emit_heads(t - 2)  # 2 steps of slack so hs is never waited on
            pg = psum_tiles.pop(t)
            pgv = pg[:, :].rearrange("p (s h b) -> p s h b", s=4, h=2, b=HB)

            sig = [None, None]
            for hb in range(2):
                for slot in range(4):
                    nc.tensor.matmul(
                        pg[:, slot * B + hb * HB:slot * B + hb * HB + HB],
                        whh_sb[:, slot * 128:(slot + 1) * 128],
                        hm_prev[hb], start=False,
                        stop=(hb == 1 and slot in (1, 3)))
                # xW fillers split so whh(h1) sits early in the PE queue
                # (only ~2 pieces behind whh(h0)), keeping the h1 chain's
                # phase offset small while the wait still has PE cover.
                emit_xw(t + 2, PIECES[0:2] if hb == 0 else PIECES[2:12])
                s = wk.tile([128, 512], bf16, tag=f"sig{hb}", name=f"sig{hb}")
                nc.scalar.activation(
                    s[:, 128:512].rearrange("p (s b) -> p s b", s=3),
                    pgv[:, 1:4, hb, :], AF.Sigmoid)
                sig[hb] = s

            # tail (per half): t2 = sig_f*c; u = (sig_g' - 0.5)*sig_i;
            # c_new = 2u + t2  (== sig_f*c + sig_i*(2*sig(2g)-1))
            cn = [None, None]
            for hb in range(2):
                t2 = wk.tile([128, HB], bf16, tag=f"t2{hb}", name=f"t2{hb}")
                nc.vector.tensor_mul(t2[:, :], sig[hb][:, 256:384], c_prev[hb])
                u = wk.tile([128, HB], bf16, tag=f"u{hb}", name=f"u{hb}")
                nc.vector.scalar_tensor_tensor(
                    u[:, :], sig[hb][:, 384:512], 0.5, sig[hb][:, 128:256],
                    OP.subtract, OP.mult)
                c_new = wk.tile([128, HB], bf16, tag=f"cn{hb}", name=f"cn{hb}")
                nc.vector.scalar_tensor_tensor(
                    c_new[:, :], u[:, :], 2.0, t2[:, :], OP.mult, OP.add)
                cn[hb] = c_new
                # off-path sigma(o) right after the chain ops are queued
                nc.scalar.activation(sig[hb][:, 0:128], pgv[:, 0, hb, :],
                                     AF.Sigmoid)

            # Pool queue gets both som's BEFORE the (slack) hs writes so
            # neither half's hm stalls behind the other's history write.
            thc = [None, None]
            for hb in range(2):
                th = wk.tile([128, HB], bf16, tag=f"th{hb}", name=f"th{hb}")
                nc.scalar.activation(th[:, :], cn[hb][:, :], AF.Tanh)
                thc[hb] = th
                if t < K - 1:
                    # som = sig_o*m on DVE, off the critical chain (runs
                    # during tanh); hm = som*tanh(c) closes the chain. Kept
                    # off Pool so the scheduler's cost model (Pool sems are
                    # ~1.4us there) doesn't predict hm late and push the
                    # next whh far back in the static PE queue.
                    som = wk.tile([128, HB], bf16, tag=f"som{hb}",
                                  name=f"som{hb}")
                    nc.vector.tensor_mul(
                        som[:, :], sig[hb][:, 0:128],
                        mt[:, mc0 + hb * HB:mc0 + hb * HB + HB])
                    hm = wk.tile([128, HB], bf16, tag=f"hm{hb}", name=f"hm{hb}")
                    nc.vector.tensor_mul(hm[:, :], som[:, :], th[:, :])
                    hm_prev[hb] = hm
                c_prev[hb] = cn[hb]
            for hb in range(2):
                # h into the bf16 history (heads-only consumer)
                col = t * B + hb * HB
                nc.vector.tensor_mul(hs_all[:, col:col + HB],
                                     sig[hb][:, 0:128], thc[hb][:, :])
        emit_heads(K - 2)
        emit_heads(K - 1)

    nc.compile()
    return nc


_NC = {}


def _get_nc(K):
    if K not in _NC:
        _NC[K] = build_nc(K)
    return _NC[K]


def _segments(done):
    """Smallest uniform span K and greedy owned ranges [(t_own0, t_own1)]
    such that every env has a reset inside each warmup window."""
    last = np.full(B, -10**9, dtype=np.int64)
    last_min = np.zeros(T, dtype=np.int64)
    for t in range(T):
        last = np.where(done[t] == 1, t, last)
        last_min[t] = last.min()
    Wt = np.arange(T) - last_min  # lookback needed at owned-start t

    def plan(K):
        end = min(K, T)
        segs = [(0, end)]
        for _ in range(1, NCORES):
            if end >= T:
                break
            t_c = end
            cap = K - Wt[t_c]
            if cap <= 0:
                return None
            end = min(t_c + cap, T)
            segs.append((t_c, end))
        if end < T:
            return None
        while len(segs) < NCORES:  # degenerate: fewer segments needed
            segs.append((T, T))
        return segs

    lo, hi = 8, T
    while lo < hi:
        mid = (lo + hi) // 2
        if plan(mid) is not None:
            hi = mid
        else:
            lo = mid + 1
    return lo, plan(lo)


def _make_in_maps(inputs, K, segs):
    import ml_dtypes

    bf16 = ml_dtypes.bfloat16
    x = np.asarray(inputs["x"], dtype=np.float32)
    done = np.asarray(inputs["done"], dtype=np.int32)
    h0 = np.asarray(inputs["h0"], dtype=np.float32).reshape(B, H)
    c0 = np.asarray(inputs["c0"], dtype=np.float32).reshape(B, H)
    Wih = np.asarray(inputs["W_ih"], dtype=np.float32)
    Whh = np.asarray(inputs["W_hh"], dtype=np.float32)
    bias = (np.asarray(inputs["b_ih"], dtype=np.float32)
            + np.asarray(inputs["b_hh"], dtype=np.float32)).reshape(4 * H)
    Wpi = np.asarray(inputs["W_pi"], dtype=np.float32)
    bpi = np.asarray(inputs["b_pi"], dtype=np.float32).reshape(A)
    Wv = np.asarray(inputs["W_v"], dtype=np.float32)
    bv = np.asarray(inputs["b_v"], dtype=np.float32).reshape(1)

    # gate order i,f,g,o -> o,i,f,g; g block (weights + bias) pre-doubled
    order = np.r_[384:512, 0:128, 128:256, 256:384]
    GS = 384  # g block offset after reorder
    FS = 256  # f block offset after reorder
    WihR = Wih[order].copy()
    WihR[GS:GS + 128] *= 2.0
    WhhR = Whh[order].copy()
    WhhR[GS:GS + 128] *= 2.0
    biasR = bias[order].copy()
    biasR[GS:GS + 128] *= 2.0

    wih_aug = np.zeros((INA, 512), dtype=np.float32)
    wih_aug[0:IN] = WihR.T
    wih_aug[IN] = biasR
    wih_aug[IN + 1, FS:FS + 128] = -30.0  # done kills the f gate
    wih_bf = wih_aug.astype(bf16)
    whh_bf = np.ascontiguousarray(WhhR.T).astype(bf16)

    wcat = np.zeros((128, 16), dtype=np.float32)
    wcat[:, 0:A] = Wpi.T
    wcat[:, A] = Wv[0]
    wcat_bf = wcat.astype(bf16)
    bhd = np.zeros((128, 32), dtype=np.float32)
    for hb in range(2):
        bhd[:, hb * 16:hb * 16 + A] = bpi
        bhd[:, hb * 16 + A] = bv[0]

    in_maps = []
    for c in range(NCORES):
        t0 = max(segs[c][1] - K, 0)  # span start (warmup-padded)
        xseg = x[t0:t0 + K]
        dseg = done[t0:t0 + K].astype(np.float32)
        xt = np.empty((INA, K * B), dtype=np.float32)
        xt[0:IN] = xseg.transpose(2, 0, 1).reshape(IN, K * B)
        xt[IN] = 1.0
        xt[IN + 1] = dseg.reshape(K * B)

        m = np.ones((K, B), dtype=np.float32)
        m[0:K - 1] = 1.0 - dseg[1:K]
        m_bc = np.ascontiguousarray(
            np.broadcast_to(m.reshape(1, K * B), (128, K * B))).astype(bf16)

        if t0 == 0:
            h0c = (h0.T * (1.0 - dseg[0])[None, :]).astype(bf16)
            c0c = np.ascontiguousarray(c0.T).astype(bf16)
        else:
            h0c = np.zeros((H, B), dtype=bf16)
            c0c = np.zeros((H, B), dtype=bf16)

        in_maps.append({
            "xt": xt.astype(bf16),
            "m": m_bc,
            "h0": np.ascontiguousarray(h0c),
            "c0": c0c,
            "wih": wih_bf,
            "whh": whh_bf,
            "wcat": wcat_bf,
            "bhd": bhd,
        })
    return in_maps


def _try_device_reset():
    try:
        import ctypes

        import jax

        jax.devices()
        lib = ctypes.CDLL("/opt/axon/libaxon_pjrt.so")
        if hasattr(lib, "axon_reset"):
            lib.axon_reset.restype = ctypes.c_int64
            lib.axon_reset()
    except Exception:
        pass


def kernel(**inputs):
    from concourse.bass_utils import run_bass_kernel_spmd

    done = np.asarray(inputs["done"], dtype=np.int32)
    K, segs = _segments(done)
    nc = _get_nc(K)
    in_maps = _make_in_maps(inputs, K, segs)
    try:
        res = run_bass_kernel_spmd(nc, in_maps, core_ids=list(range(NCORES)))
    except Exception:
        _try_device_reset()
        res = run_bass_kernel_spmd(nc, in_maps, core_ids=list(range(NCORES)))
    outs = [r["out"].reshape(K, B, 16)[:, :, 0:NOUT] for r in res.results]
    full = np.empty((T, B, NOUT), dtype=np.float32)
    for c in range(NCORES):
        o0, o1 = segs[c]
        if o1 <= o0:
            continue
        t0 = max(o1 - K, 0)
        full[o0:o1] = outs[c][o0 - t0:o1 - t0]
    return full.reshape(T * B, NOUT).copy()
